# revision 1
# baseline (speedup 1.0000x reference)
"""Causal self-attention layer (B=4, T=2048, C=1024, H=16) on 8 TRN2 NeuronCores.

Sharding: Megatron-style tensor parallel over heads — 2 heads per core.
Each core computes q/k/v projections for its 2 heads (transposed layouts,
fp32r matmuls on the PE at full rate), causal flash-style attention with an
appended-ones column on V to accumulate softmax denominators, and a partial
output projection against its 128-row slice of W_proj. The host sums the 8
partial projections and adds b_proj.

All matmul operands are float32r (fp32 data pre-rounded by on-chip compute
copies — raw unrounded fp32 fed to an fp32r matmul is fatal on HW).

Scheduling notes (measured on HW): matmuls inside a PSUM accumulation group
stream at ~227 ns per 512-col instruction, while every group boundary
(start/stop) adds ~205 ns of pipeline drain. So q/k/v projections are emitted
as three consecutive 8-matmul groups per token tile, the attention inner loop
emits score singles and the grouped P@V accumulation in separate runs, and
softmax denominators are normalized with one wide reciprocal per batch
instead of per-row [1,512] reciprocals (3.3 us each on a single DVE lane).
The output projection is interleaved per batch to hide the 32 MB output DMA.
"""
import sys

sys.path.insert(0, "/opt/trn_rl_repo")

import numpy as np

import concourse.bass as bass  # noqa: F401
from concourse import bacc
import concourse.mybir as mybir
import concourse.tile as tile
from concourse.bass_utils import run_bass_kernel_spmd
from concourse.masks import make_identity

B, T, C = 4, 2048, 1024
H, DH = 16, 64
N_CORES = 8
HPC = H // N_CORES          # heads per core = 2
DPC = HPC * DH              # head-dims per core = 128
NT = B * T                  # 8192 tokens
CH = C // 128               # 8 contraction chunks
QB = 512                    # q-block width (moving dim, >=256 for f32r full rate)
KT = 128                    # k-tile width (PE partition dim)
CHUNK = 8                   # k-tiles per S/PV emission chunk
SCALE = 1.0 / 8.0           # 1/sqrt(DH)

F32 = mybir.dt.float32
F32R = mybir.dt.float32r
AF = mybir.ActivationFunctionType

_CACHED_NC = None
LAST_RESULT = None


def _build():
    nc = bacc.Bacc(None)

    xT = nc.dram_tensor("xT", [C, NT], F32, kind="ExternalInput")
    wq = nc.dram_tensor("wq", [C, DPC], F32, kind="ExternalInput")
    wk = nc.dram_tensor("wk", [C, DPC], F32, kind="ExternalInput")
    wv = nc.dram_tensor("wv", [C, DPC], F32, kind="ExternalInput")
    bq = nc.dram_tensor("bq", [DPC, 1], F32, kind="ExternalInput")
    bk = nc.dram_tensor("bk", [DPC, 1], F32, kind="ExternalInput")
    bv = nc.dram_tensor("bv", [DPC, 1], F32, kind="ExternalInput")
    wp = nc.dram_tensor("wp", [DPC, C], F32, kind="ExternalInput")
    emat_in = nc.dram_tensor("emat", [8, 4, 128], F32, kind="ExternalInput")
    out = nc.dram_tensor("out", [NT, C], F32, kind="ExternalOutput")

    with tile.TileContext(nc) as tc:
        with (
            tc.tile_pool(name="const", bufs=1) as const,
            tc.tile_pool(name="res", bufs=1) as res,
        ):
            # --- constants (built in f32, converted: f32r rejects memset) ---
            # two stacked 64x64 identities so the transpose identity operand
            # can share base partition (0 or 64) with the vT slice
            ident = const.tile([128, DH], F32R, tag="ident")
            # sliding causal mask: wmask[k, u] = 1 iff k <= u - 512; a crossing
            # tile r multiplies by wmask[:, 512-128r : 1024-128r]
            wmask = const.tile([128, 1024], F32R, tag="wmask")
            ones_col = const.tile([128, 1], F32R, tag="ones_col")
            # indicator lhsT per q-block: Emat[:, qb, j] selects den row qb
            # (head 0) for j<64 and row 4+qb (head 1) for j>=64, so one matmul
            # broadcasts both heads' reciprocals into a [128, 512] tile
            emat = const.tile([8, 4, 128], F32R, tag="emat")
            with tc.tile_pool(name="cstage", bufs=1) as cstage:
                ident_s = cstage.tile([128, DH], F32, tag="ident_s")
                make_identity(nc, ident_s[:DH, :])
                make_identity(nc, ident_s[DH:, :])
                nc.vector.tensor_copy(ident[:], ident_s[:])

                wmask_s = cstage.tile([128, 1024], F32, tag="wmask_s")
                nc.gpsimd.memset(wmask_s[:], 0.0)
                nc.gpsimd.affine_select(
                    out=wmask_s[:],
                    in_=wmask_s[:],
                    compare_op=mybir.AluOpType.is_gt,
                    fill=1.0,
                    base=512,
                    # keep 0 where (512 + k - u) > 0, fill 1 where k <= u - 512
                    pattern=[[-1, 1024]],
                    channel_multiplier=1,
                )
                nc.vector.tensor_copy(wmask[:], wmask_s[:])

                ones_s = cstage.tile([128, DH], F32, tag="ones_s")
                nc.gpsimd.memset(ones_s[:], 1.0)
                nc.vector.tensor_copy(ones_col[:], ones_s[:, :1])

                emat_s = cstage.tile([8, 4, 128], F32, tag="emat_s")
                nc.sync.dma_start(emat_s[:], emat_in[:])
                nc.vector.tensor_copy(emat[:], emat_s[:])

            bq_t = const.tile([DPC, 1], F32, tag="bq")
            bk_t = const.tile([DPC, 1], F32, tag="bk")
            bv_t = const.tile([DPC, 1], F32, tag="bv")
            nc.sync.dma_start(bq_t[:], bq[:])
            nc.sync.dma_start(bk_t[:], bk[:])
            nc.sync.dma_start(bv_t[:], bv[:])

            # weights -> SBUF, converted (rounded) to f32r
            wq_r = const.tile([128, CH, DPC], F32R, tag="wq_r")
            wk_r = const.tile([128, CH, DPC], F32R, tag="wk_r")
            wv_r = const.tile([128, CH, DPC], F32R, tag="wv_r")
            wp_r = const.tile([DPC, C], F32R, tag="wp_r")
            with tc.tile_pool(name="wstage", bufs=3) as wstage:
                for w_in, w_dst in ((wq, wq_r), (wk, wk_r), (wv, wv_r)):
                    w_re = w_in.rearrange("(c p) n -> c p n", p=128)
                    for c in range(CH):
                        ws = wstage.tile([128, DPC], F32, tag="ws")
                        nc.sync.dma_start(ws[:], w_re[c])
                        nc.vector.tensor_copy(w_dst[:, c, :], ws[:])
                for half in range(2):
                    ws = wstage.tile([DPC, 512], F32, tag="wsp")
                    nc.sync.dma_start(ws[:], wp[:, half * 512 : (half + 1) * 512])
                    nc.vector.tensor_copy(
                        wp_r[:, half * 512 : (half + 1) * 512], ws[:]
                    )

            # --- residents ---
            qT = res.tile([DPC, NT], F32R, tag="qT")
            kT = res.tile([DPC, NT], F32R, tag="kT")
            vT = res.tile([DPC, NT], F32R, tag="vT")
            yT = res.tile([DPC, NT], F32R, tag="yT")

            # ================= phase 1: q/k/v projections =================
            xT_re = xT.rearrange("(c p) t -> c p t", p=128)
            with (
                tc.tile_pool(name="xpool", bufs=4) as xpool,
                tc.tile_pool(name="xrpool", bufs=10) as xrpool,
                tc.tile_pool(name="qkv_psum", bufs=2, space="PSUM") as qkv_psum,
            ):
                for tt in range(NT // QB):
                    ts_ = slice(tt * QB, (tt + 1) * QB)
                    xrs = []
                    for c in range(CH):
                        xs = xpool.tile([128, QB], F32, tag="xs")
                        nc.sync.dma_start(xs[:], xT_re[c, :, ts_])
                        xr = xrpool.tile([128, QB], F32R, tag="xr", name=f"xr{tt}_{c}")
                        nc.vector.tensor_copy(xr[:], xs[:])
                        xrs.append(xr)
                    # three consecutive 8-matmul accumulation groups
                    psq = qkv_psum.tile([128, QB], F32, tag="psq")
                    psk = qkv_psum.tile([128, QB], F32, tag="psk")
                    psv = qkv_psum.tile([128, QB], F32, tag="psv")
                    for ps, w_r in ((psq, wq_r), (psk, wk_r), (psv, wv_r)):
                        for c in range(CH):
                            nc.tensor.matmul(
                                ps[:], w_r[:, c, :], xrs[c][:],
                                start=(c == 0), stop=(c == CH - 1),
                            )
                    # copy out of PSUM (+bias; q also scaled by 1/sqrt(dh))
                    nc.scalar.activation(qT[:, ts_], psq[:], AF.Identity, bias=bq_t[:], scale=SCALE)
                    nc.scalar.activation(kT[:, ts_], psk[:], AF.Identity, bias=bk_t[:])
                    nc.vector.tensor_scalar_add(vT[:, ts_], psv[:], bv_t[:])

            # ============ phase 2+3: attention + proj, per batch ==========
            with (
                tc.tile_pool(name="vpool", bufs=18) as vpool,
                tc.tile_pool(name="epool", bufs=CHUNK + 3) as epool,
                tc.tile_pool(name="dpool", bufs=1) as dpool,
                tc.tile_pool(name="opool", bufs=4) as opool,
                tc.tile_pool(name="s_psum", bufs=2, space="PSUM") as s_psum,
                tc.tile_pool(name="y_psum", bufs=2, space="PSUM") as y_psum,
                tc.tile_pool(name="m_psum", bufs=2, space="PSUM") as m_psum,
                tc.tile_pool(name="p_psum", bufs=2, space="PSUM") as p_psum,
            ):
                n_ktiles = T // KT  # 16
                for b in range(B):
                    cb = b * T
                    # denominator rows accumulate on one partition (compute
                    # engines can only write partition bases 0/32/64/96), then
                    # one SBUF->SBUF DMA scatters them across 8 partitions
                    denw = dpool.tile([1, 8 * QB], F32, tag="denw", name=f"denw{b}")
                    den = dpool.tile([8, QB], F32, tag="den", name=f"den{b}")
                    for hl in range(HPC):
                        rb = hl * DH
                        # V tiles [128 tok, DH+1] with an appended ones column
                        vts = []
                        for kt in range(n_ktiles):
                            pt = m_psum.tile([128, 512], F32R, tag="mps", name=f"pt{b}_{hl}_{kt}")
                            nc.tensor.transpose(
                                pt[:, :DH],
                                vT[rb : rb + DH, cb + kt * KT : cb + (kt + 1) * KT],
                                ident[rb : rb + DH, :],
                            )
                            v = vpool.tile([128, DH + 1], F32R, tag="v", name=f"v{b}_{hl}_{kt}")
                            nc.vector.tensor_copy(v[:, :DH], pt[:, :DH])
                            nc.vector.tensor_copy(v[:, DH : DH + 1], ones_col[:])
                            vts.append(v)
                        for qb in range(T // QB):
                            qs = slice(cb + qb * QB, cb + (qb + 1) * QB)
                            py = y_psum.tile([128, QB], F32, tag="py", name=f"py{b}_{hl}_{qb}")
                            nkt = (qb + 1) * (QB // KT)
                            for k0 in range(0, nkt, CHUNK):
                                kts = range(k0, min(k0 + CHUNK, nkt))
                                exs = {}
                                # scores + exp for this chunk
                                for kt in kts:
                                    ps = s_psum.tile([128, QB], F32, tag="ps", name=f"ps{kt}")
                                    nc.tensor.matmul(
                                        ps[:],
                                        kT[rb : rb + DH, cb + kt * KT : cb + (kt + 1) * KT],
                                        qT[rb : rb + DH, qs],
                                        start=True,
                                        stop=True,
                                    )
                                    ex = epool.tile([128, QB], F32R, tag="ex", name=f"ex{kt}")
                                    nc.scalar.activation(ex[:], ps[:], AF.Exp)
                                    r = kt - qb * (QB // KT)
                                    if r >= 0:
                                        # diagonal-crossing tile: zero out k > q
                                        nc.vector.tensor_mul(
                                            ex[:], ex[:],
                                            wmask[:, 512 - r * KT : 1024 - r * KT],
                                        )
                                    exs[kt] = ex
                                # grouped P@V accumulation for this chunk
                                for kt in kts:
                                    nc.tensor.matmul(
                                        py[: DH + 1],
                                        vts[kt][:],
                                        exs[kt][:],
                                        start=(kt == 0),
                                        stop=(kt == nkt - 1),
                                    )
                            # stash unnormalized y and the denominator row
                            p = hl * 4 + qb
                            nc.vector.tensor_copy(
                                denw[:, p * QB : (p + 1) * QB], py[DH : DH + 1, :]
                            )
                            nc.vector.tensor_copy(yT[rb : rb + DH, qs], py[:DH, :])
                    # scatter to 8 partitions (plain APs so the dependency
                    # tracker sees exact regions), then one wide reciprocal
                    for p in range(8):
                        nc.sync.dma_start(
                            den[p : p + 1, :], denw[:, p * QB : (p + 1) * QB]
                        )
                    rec = dpool.tile([8, QB], F32R, tag="rec", name=f"rec{b}")
                    with nc.allow_low_precision(reason="softmax denom recip to f32r"):
                        nc.vector.reciprocal(rec[:], den[:])
                    for qb in range(T // QB):
                        qs = slice(cb + qb * QB, cb + (qb + 1) * QB)
                        pb = m_psum.tile([128, 512], F32, tag="mps", name=f"pb{b}_{qb}")
                        nc.tensor.matmul(
                            pb[:, :QB], emat[:, qb, :], rec[:],
                            start=True, stop=True,
                        )
                        nc.vector.tensor_mul(yT[:, qs], yT[:, qs], pb[:, :QB])
                    # partial output projection for this batch's tokens
                    for tt in range(T // 128):
                        trow = cb + tt * 128
                        for half in range(2):
                            pp = p_psum.tile([128, 512], F32, tag="pp", name=f"pp{b}_{tt}_{half}")
                            nc.tensor.matmul(
                                pp[:],
                                yT[:, trow : trow + 128],
                                wp_r[:, half * 512 : (half + 1) * 512],
                                start=True,
                                stop=True,
                            )
                            os_ = opool.tile([128, 512], F32, tag="os", name=f"os{b}_{tt}_{half}")
                            # alternate copy engine: ACT is exp-loaded, DVE has slack
                            if (tt + half) % 2 == 0:
                                nc.vector.tensor_copy(os_[:], pp[:])
                            else:
                                nc.scalar.copy(os_[:], pp[:])
                            nc.sync.dma_start(
                                out[trow : trow + 128, half * 512 : (half + 1) * 512],
                                os_[:],
                            )

    nc.compile()
    return nc


def _get_nc():
    global _CACHED_NC
    if _CACHED_NC is None:
        _CACHED_NC = _build()
    return _CACHED_NC


def kernel(x, W_qkv, b_qkv, W_proj, b_proj, _trace=False, _core_ids=None):
    global LAST_RESULT
    x = np.asarray(x, dtype=np.float32)
    W_qkv = np.asarray(W_qkv, dtype=np.float32)
    b_qkv = np.asarray(b_qkv, dtype=np.float32)
    W_proj = np.asarray(W_proj, dtype=np.float32)
    b_proj = np.asarray(b_proj, dtype=np.float32)

    xT = np.ascontiguousarray(x.reshape(NT, C).T)
    emat_np = np.zeros((8, 4, 128), dtype=np.float32)
    for qb in range(4):
        emat_np[qb, qb, :DH] = 1.0
        emat_np[4 + qb, qb, DH:] = 1.0
    core_ids = list(range(N_CORES)) if _core_ids is None else _core_ids
    in_maps = []
    for core in range(len(core_ids)):
        s = slice(core * DPC, (core + 1) * DPC)
        in_maps.append(
            {
                "xT": xT,
                "wq": np.ascontiguousarray(W_qkv[:, 0 * C + core * DPC : 0 * C + (core + 1) * DPC]),
                "wk": np.ascontiguousarray(W_qkv[:, 1 * C + core * DPC : 1 * C + (core + 1) * DPC]),
                "wv": np.ascontiguousarray(W_qkv[:, 2 * C + core * DPC : 2 * C + (core + 1) * DPC]),
                # device computes qT = psq*SCALE + bias, so pre-scale the q bias
                "bq": np.ascontiguousarray(b_qkv[0 * C + core * DPC : 0 * C + (core + 1) * DPC, None]) * np.float32(SCALE),
                "bk": np.ascontiguousarray(b_qkv[1 * C + core * DPC : 1 * C + (core + 1) * DPC, None]),
                "bv": np.ascontiguousarray(b_qkv[2 * C + core * DPC : 2 * C + (core + 1) * DPC, None]),
                "wp": np.ascontiguousarray(W_proj[s, :]),
                "emat": emat_np,
            }
        )

    nc = _get_nc()
    res = run_bass_kernel_spmd(nc, in_maps, core_ids, trace=_trace)
    LAST_RESULT = res

    acc = np.zeros((NT, C), dtype=np.float64)
    for r in res.results:
        acc += r["out"].astype(np.float64)
    acc += b_proj.astype(np.float64)
    return acc.reshape(B, T, C).astype(np.float32)



# revision 3
# speedup vs baseline: 1.3488x; 1.3488x over previous
"""Causal self-attention layer (B=4, T=2048, C=1024, H=16) on 8 TRN2 NeuronCores.

Sharding: Megatron-style tensor parallel over heads — 2 heads per core.
Each core computes q/k/v projections for its 2 heads, causal flash-style
attention with an appended-ones column on V to accumulate softmax
denominators, and a partial output projection against its 128-row slice of
W_proj. The host sums the 8 partial projections and adds b_proj.

All matmul operands are bfloat16 (pre-cast on host for x/weights; on-chip
activations write bf16 directly out of PSUM). fp32r matmuls run in
fp32_mode=HIGH which power-throttles the PE to 50% for ~2/3 of the kernel
(ham duty cycle 34us full / 72us half); bf16 keeps the PE at full rate,
halves the x and out DMA bytes, and doubles DVE throughput on the mask
multiplies.

Scheduling notes (measured on HW): matmuls stream back-to-back at ~233 ns
per 512-col instruction when PSUM banks rotate; q/k/v projections are
emitted as three consecutive 8-matmul accumulation groups per token tile,
the attention inner loop emits score singles and the grouped P@V
accumulation in separate runs, and softmax denominators are normalized with
one wide reciprocal per batch. The output projection is interleaved per
batch to hide the output DMA.
"""
import sys

sys.path.insert(0, "/opt/trn_rl_repo")

import numpy as np
import ml_dtypes

import concourse.bass as bass  # noqa: F401
from concourse import bacc
import concourse.mybir as mybir
import concourse.tile as tile
from concourse.bass_utils import run_bass_kernel_spmd
from concourse.masks import make_identity

B, T, C = 4, 2048, 1024
H, DH = 16, 64
N_CORES = 8
HPC = H // N_CORES          # heads per core = 2
DPC = HPC * DH              # head-dims per core = 128
NT = B * T                  # 8192 tokens
CH = C // 128               # 8 contraction chunks
QB = 512                    # q-block width (moving dim)
KT = 128                    # k-tile width (PE partition dim)
CHUNK = 8                   # k-tiles per S/PV emission chunk
SCALE = 1.0 / 8.0           # 1/sqrt(DH)

F32 = mybir.dt.float32
BF16 = mybir.dt.bfloat16
AF = mybir.ActivationFunctionType
BF16_NP = ml_dtypes.bfloat16

_CACHED_NC = None
LAST_RESULT = None


def _build():
    nc = bacc.Bacc(None)

    xT = nc.dram_tensor("xT", [C, NT], BF16, kind="ExternalInput")
    wq = nc.dram_tensor("wq", [C, DPC], BF16, kind="ExternalInput")
    wk = nc.dram_tensor("wk", [C, DPC], BF16, kind="ExternalInput")
    wv = nc.dram_tensor("wv", [C, DPC], BF16, kind="ExternalInput")
    bq = nc.dram_tensor("bq", [DPC, 1], F32, kind="ExternalInput")
    bk = nc.dram_tensor("bk", [DPC, 1], F32, kind="ExternalInput")
    bv = nc.dram_tensor("bv", [DPC, 1], F32, kind="ExternalInput")
    wp = nc.dram_tensor("wp", [DPC, C], BF16, kind="ExternalInput")
    emat_in = nc.dram_tensor("emat", [8, 4, 128], BF16, kind="ExternalInput")
    out = nc.dram_tensor("out", [NT, C], BF16, kind="ExternalOutput")

    with tile.TileContext(nc) as tc:
        with (
            tc.tile_pool(name="const", bufs=1) as const,
            tc.tile_pool(name="res", bufs=1) as res,
        ):
            # --- constants (built in f32, cast to bf16 once) ---
            # two stacked 64x64 identities so the transpose identity operand
            # can share base partition (0 or 64) with the vT slice
            ident = const.tile([128, DH], BF16, tag="ident")
            # sliding causal mask: wmask[k, u] = 1 iff k <= u - 512; a crossing
            # tile r multiplies by wmask[:, 512-128r : 1024-128r]
            wmask = const.tile([128, 1024], BF16, tag="wmask")
            ones_col = const.tile([128, 1], BF16, tag="ones_col")
            # indicator lhsT per q-block: Emat[:, qb, j] selects den row qb
            # (head 0) for j<64 and row 4+qb (head 1) for j>=64, so one matmul
            # broadcasts both heads' reciprocals into a [128, 512] tile
            emat = const.tile([8, 4, 128], BF16, tag="emat")
            with tc.tile_pool(name="cstage", bufs=1) as cstage:
                ident_s = cstage.tile([128, DH], F32, tag="ident_s")
                make_identity(nc, ident_s[:DH, :])
                make_identity(nc, ident_s[DH:, :])
                nc.vector.tensor_copy(ident[:], ident_s[:])

                wmask_s = cstage.tile([128, 1024], F32, tag="wmask_s")
                nc.gpsimd.memset(wmask_s[:], 0.0)
                nc.gpsimd.affine_select(
                    out=wmask_s[:],
                    in_=wmask_s[:],
                    compare_op=mybir.AluOpType.is_gt,
                    fill=1.0,
                    base=512,
                    # keep 0 where (512 + k - u) > 0, fill 1 where k <= u - 512
                    pattern=[[-1, 1024]],
                    channel_multiplier=1,
                )
                nc.vector.tensor_copy(wmask[:], wmask_s[:])

                ones_s = cstage.tile([128, DH], F32, tag="ones_s")
                nc.gpsimd.memset(ones_s[:], 1.0)
                nc.vector.tensor_copy(ones_col[:], ones_s[:, :1])

            bq_t = const.tile([DPC, 1], F32, tag="bq")
            bk_t = const.tile([DPC, 1], F32, tag="bk")
            bv_t = const.tile([DPC, 1], F32, tag="bv")
            nc.sync.dma_start(bq_t[:], bq[:])
            nc.sync.dma_start(bk_t[:], bk[:])
            nc.sync.dma_start(bv_t[:], bv[:])

            # weights -> SBUF directly in bf16 (cast on host)
            wq_r = const.tile([128, CH, DPC], BF16, tag="wq_r")
            wk_r = const.tile([128, CH, DPC], BF16, tag="wk_r")
            wv_r = const.tile([128, CH, DPC], BF16, tag="wv_r")
            wp_r = const.tile([DPC, C], BF16, tag="wp_r")
            for w_in, w_dst in ((wq, wq_r), (wk, wk_r), (wv, wv_r)):
                w_re = w_in.rearrange("(c p) n -> p c n", p=128)
                nc.sync.dma_start(w_dst[:], w_re[:])
            nc.sync.dma_start(wp_r[:], wp[:])
            nc.sync.dma_start(emat[:], emat_in[:])

            # --- residents ---
            qT = res.tile([DPC, NT], BF16, tag="qT")
            kT = res.tile([DPC, NT], BF16, tag="kT")
            vT = res.tile([DPC, NT], BF16, tag="vT")
            yT = res.tile([DPC, NT], BF16, tag="yT")

            # ================= phase 1: q/k/v projections =================
            xT_re = xT.rearrange("(c p) t -> c p t", p=128)
            with (
                tc.tile_pool(name="xpool", bufs=6) as xpool,
                tc.tile_pool(name="qkv_psum", bufs=2, space="PSUM") as qkv_psum,
            ):
                for tt in range(NT // QB):
                    ts_ = slice(tt * QB, (tt + 1) * QB)
                    xrs = []
                    for c in range(CH):
                        xs = xpool.tile([128, QB], BF16, tag="xs", name=f"xs{tt}_{c}")
                        nc.sync.dma_start(xs[:], xT_re[c, :, ts_])
                        xrs.append(xs)
                    # three consecutive 8-matmul accumulation groups
                    psq = qkv_psum.tile([128, QB], F32, tag="psq")
                    psk = qkv_psum.tile([128, QB], F32, tag="psk")
                    psv = qkv_psum.tile([128, QB], F32, tag="psv")
                    for ps, w_r in ((psq, wq_r), (psk, wk_r), (psv, wv_r)):
                        for c in range(CH):
                            nc.tensor.matmul(
                                ps[:], w_r[:, c, :], xrs[c][:],
                                start=(c == 0), stop=(c == CH - 1),
                            )
                    # copy out of PSUM (+bias; q also scaled by 1/sqrt(dh))
                    nc.scalar.activation(qT[:, ts_], psq[:], AF.Identity, bias=bq_t[:], scale=SCALE)
                    nc.scalar.activation(kT[:, ts_], psk[:], AF.Identity, bias=bk_t[:])
                    nc.vector.tensor_scalar_add(vT[:, ts_], psv[:], bv_t[:])

            # ============ phase 2+3: attention + proj, per batch ==========
            with (
                tc.tile_pool(name="vpool", bufs=18) as vpool,
                tc.tile_pool(name="epool", bufs=CHUNK + 3) as epool,
                tc.tile_pool(name="dpool", bufs=1) as dpool,
                tc.tile_pool(name="opool", bufs=4) as opool,
                tc.tile_pool(name="s_psum", bufs=2, space="PSUM") as s_psum,
                tc.tile_pool(name="y_psum", bufs=2, space="PSUM") as y_psum,
                tc.tile_pool(name="m_psum", bufs=2, space="PSUM") as m_psum,
                tc.tile_pool(name="p_psum", bufs=2, space="PSUM") as p_psum,
            ):
                n_ktiles = T // KT  # 16
                for b in range(B):
                    cb = b * T
                    # denominator rows accumulate on one partition (compute
                    # engines can only write partition bases 0/32/64/96), then
                    # one SBUF->SBUF DMA scatters them across 8 partitions
                    denw = dpool.tile([1, 8 * QB], F32, tag="denw", name=f"denw{b}")
                    den = dpool.tile([8, QB], F32, tag="den", name=f"den{b}")
                    for hl in range(HPC):
                        rb = hl * DH
                        # V tiles [128 tok, DH+1] with an appended ones column
                        vts = []
                        for kt in range(n_ktiles):
                            pt = m_psum.tile([128, 512], BF16, tag="mps", name=f"pt{b}_{hl}_{kt}")
                            nc.tensor.transpose(
                                pt[:, :DH],
                                vT[rb : rb + DH, cb + kt * KT : cb + (kt + 1) * KT],
                                ident[rb : rb + DH, :],
                            )
                            v = vpool.tile([128, DH + 1], BF16, tag="v", name=f"v{b}_{hl}_{kt}")
                            nc.vector.tensor_copy(v[:, :DH], pt[:, :DH])
                            nc.vector.tensor_copy(v[:, DH : DH + 1], ones_col[:])
                            vts.append(v)
                        for qb in range(T // QB):
                            qs = slice(cb + qb * QB, cb + (qb + 1) * QB)
                            py = y_psum.tile([128, QB], F32, tag="py", name=f"py{b}_{hl}_{qb}")
                            nkt = (qb + 1) * (QB // KT)
                            for k0 in range(0, nkt, CHUNK):
                                kts = range(k0, min(k0 + CHUNK, nkt))
                                exs = {}
                                # scores + exp for this chunk
                                for kt in kts:
                                    ps = s_psum.tile([128, QB], F32, tag="ps", name=f"ps{kt}")
                                    nc.tensor.matmul(
                                        ps[:],
                                        kT[rb : rb + DH, cb + kt * KT : cb + (kt + 1) * KT],
                                        qT[rb : rb + DH, qs],
                                        start=True,
                                        stop=True,
                                    )
                                    ex = epool.tile([128, QB], BF16, tag="ex", name=f"ex{kt}")
                                    nc.scalar.activation(ex[:], ps[:], AF.Exp)
                                    r = kt - qb * (QB // KT)
                                    if r >= 0:
                                        # diagonal-crossing tile: zero out k > q
                                        nc.vector.tensor_mul(
                                            ex[:], ex[:],
                                            wmask[:, 512 - r * KT : 1024 - r * KT],
                                        )
                                    exs[kt] = ex
                                # grouped P@V accumulation for this chunk
                                for kt in kts:
                                    nc.tensor.matmul(
                                        py[: DH + 1],
                                        vts[kt][:],
                                        exs[kt][:],
                                        start=(kt == 0),
                                        stop=(kt == nkt - 1),
                                    )
                            # stash unnormalized y and the denominator row
                            p = hl * 4 + qb
                            nc.vector.tensor_copy(
                                denw[:, p * QB : (p + 1) * QB], py[DH : DH + 1, :]
                            )
                            nc.vector.tensor_copy(yT[rb : rb + DH, qs], py[:DH, :])
                    # scatter to 8 partitions (plain APs so the dependency
                    # tracker sees exact regions), then one wide reciprocal
                    for p in range(8):
                        nc.sync.dma_start(
                            den[p : p + 1, :], denw[:, p * QB : (p + 1) * QB]
                        )
                    rec = dpool.tile([8, QB], BF16, tag="rec", name=f"rec{b}")
                    with nc.allow_low_precision(reason="softmax denom recip to bf16"):
                        nc.vector.reciprocal(rec[:], den[:])
                    for qb in range(T // QB):
                        qs = slice(cb + qb * QB, cb + (qb + 1) * QB)
                        pb = m_psum.tile([128, 512], F32, tag="mps", name=f"pb{b}_{qb}")
                        nc.tensor.matmul(
                            pb[:, :QB], emat[:, qb, :], rec[:],
                            start=True, stop=True,
                        )
                        nc.vector.tensor_mul(yT[:, qs], yT[:, qs], pb[:, :QB])
                    # partial output projection for this batch's tokens
                    for tt in range(T // 128):
                        trow = cb + tt * 128
                        for half in range(2):
                            pp = p_psum.tile([128, 512], F32, tag="pp", name=f"pp{b}_{tt}_{half}")
                            nc.tensor.matmul(
                                pp[:],
                                yT[:, trow : trow + 128],
                                wp_r[:, half * 512 : (half + 1) * 512],
                                start=True,
                                stop=True,
                            )
                            os_ = opool.tile([128, 512], BF16, tag="os", name=f"os{b}_{tt}_{half}")
                            # alternate copy engine: ACT is exp-loaded, DVE has slack
                            if (tt + half) % 2 == 0:
                                nc.vector.tensor_copy(os_[:], pp[:])
                            else:
                                nc.scalar.copy(os_[:], pp[:])
                            nc.sync.dma_start(
                                out[trow : trow + 128, half * 512 : (half + 1) * 512],
                                os_[:],
                            )

    nc.compile()
    return nc


def _get_nc():
    global _CACHED_NC
    if _CACHED_NC is None:
        _CACHED_NC = _build()
    return _CACHED_NC


def kernel(x, W_qkv, b_qkv, W_proj, b_proj, _trace=False, _core_ids=None):
    global LAST_RESULT
    x = np.asarray(x, dtype=np.float32)
    W_qkv = np.asarray(W_qkv, dtype=np.float32)
    b_qkv = np.asarray(b_qkv, dtype=np.float32)
    W_proj = np.asarray(W_proj, dtype=np.float32)
    b_proj = np.asarray(b_proj, dtype=np.float32)

    xT = np.ascontiguousarray(x.reshape(NT, C).T).astype(BF16_NP)
    W_qkv_b = W_qkv.astype(BF16_NP)
    W_proj_b = W_proj.astype(BF16_NP)
    emat_np = np.zeros((8, 4, 128), dtype=BF16_NP)
    for qb in range(4):
        emat_np[qb, qb, :DH] = 1.0
        emat_np[4 + qb, qb, DH:] = 1.0
    core_ids = list(range(N_CORES)) if _core_ids is None else _core_ids
    in_maps = []
    for core in range(len(core_ids)):
        s = slice(core * DPC, (core + 1) * DPC)
        in_maps.append(
            {
                "xT": xT,
                "wq": np.ascontiguousarray(W_qkv_b[:, 0 * C + core * DPC : 0 * C + (core + 1) * DPC]),
                "wk": np.ascontiguousarray(W_qkv_b[:, 1 * C + core * DPC : 1 * C + (core + 1) * DPC]),
                "wv": np.ascontiguousarray(W_qkv_b[:, 2 * C + core * DPC : 2 * C + (core + 1) * DPC]),
                # device computes qT = psq*SCALE + bias, so pre-scale the q bias
                "bq": np.ascontiguousarray(b_qkv[0 * C + core * DPC : 0 * C + (core + 1) * DPC, None]) * np.float32(SCALE),
                "bk": np.ascontiguousarray(b_qkv[1 * C + core * DPC : 1 * C + (core + 1) * DPC, None]),
                "bv": np.ascontiguousarray(b_qkv[2 * C + core * DPC : 2 * C + (core + 1) * DPC, None]),
                "wp": np.ascontiguousarray(W_proj_b[s, :]),
                "emat": emat_np,
            }
        )

    nc = _get_nc()
    res = run_bass_kernel_spmd(nc, in_maps, core_ids, trace=_trace)
    LAST_RESULT = res

    acc = np.zeros((NT, C), dtype=np.float64)
    for r in res.results:
        acc += r["out"].astype(np.float64)
    acc += b_proj.astype(np.float64)
    return acc.reshape(B, T, C).astype(np.float32)


# revision 6
# speedup vs baseline: 1.4817x; 1.0986x over previous
"""Causal self-attention layer (B=4, T=2048, C=1024, H=16) on 8 TRN2 NeuronCores.

Sharding: Megatron-style tensor parallel over heads — 2 heads per core.
Each core computes q/k/v projections for its 2 heads, causal flash-style
attention with ones-columns on V to accumulate softmax denominators, and a
partial output projection against its 128-row slice of W_proj. The host sums
the 8 partial projections and adds b_proj.

All matmul operands are bfloat16 (pre-cast on host for x/weights; on-chip
activations write bf16 directly out of PSUM). fp32r matmuls run in
fp32_mode=HIGH which power-throttles the PE to 50% for most of the kernel;
bf16 keeps the PE mostly at full rate, halves the x/out DMA bytes, and
doubles DVE throughput on the mask multiplies.

Schedule: the q/k/v projection tiles for batch b+1 are interleaved into the
attention stream of batch b, so the PE never runs a long unbroken stream of
dense projection matmuls (which exhausts the power budget and triggers the
50%-duty throttle), and no phase boundary stalls the PE. V tiles are
transposed once per batch ([128,128] both-heads transpose) with two ones
columns (at free offsets 64 and 129) so each head's 65-wide PV lhsT slice
puts y in psum rows 0-63 and the softmax denominator in row 64. The V
transposes for batch b+1 are emitted in the denominator-reciprocal latency
gap of batch b. PSUM: 4 pools x 2 bufs x 2KB = exactly 8 banks.
"""
import sys

sys.path.insert(0, "/opt/trn_rl_repo")

import numpy as np
import ml_dtypes

import concourse.bass as bass  # noqa: F401
from concourse import bacc
import concourse.mybir as mybir
import concourse.tile as tile
from concourse.bass_utils import run_bass_kernel_spmd
from concourse.masks import make_identity

B, T, C = 4, 2048, 1024
H, DH = 16, 64
N_CORES = 8
HPC = H // N_CORES          # heads per core = 2
DPC = HPC * DH              # head-dims per core = 128
NT = B * T                  # 8192 tokens
CH = C // 128               # 8 contraction chunks
QB = 512                    # q-block width (moving dim)
KT = 128                    # k-tile width (PE partition dim)
CHUNK = 8                   # k-tiles per S/PV emission chunk
SCALE = 1.0 / 8.0           # 1/sqrt(DH)
TPB = T // QB               # qkv token tiles per batch = 4

F32 = mybir.dt.float32
BF16 = mybir.dt.bfloat16
AF = mybir.ActivationFunctionType
BF16_NP = ml_dtypes.bfloat16

_CACHED_NC = None
LAST_RESULT = None


def _build():
    nc = bacc.Bacc(None)

    xT = nc.dram_tensor("xT", [C, NT], BF16, kind="ExternalInput")
    wq = nc.dram_tensor("wq", [C, DPC], BF16, kind="ExternalInput")
    wk = nc.dram_tensor("wk", [C, DPC], BF16, kind="ExternalInput")
    wv = nc.dram_tensor("wv", [C, DPC], BF16, kind="ExternalInput")
    bq = nc.dram_tensor("bq", [DPC, 1], F32, kind="ExternalInput")
    bk = nc.dram_tensor("bk", [DPC, 1], F32, kind="ExternalInput")
    bv = nc.dram_tensor("bv", [DPC, 1], F32, kind="ExternalInput")
    wp = nc.dram_tensor("wp", [DPC, C], BF16, kind="ExternalInput")
    emat_in = nc.dram_tensor("emat", [8, 4, 128], BF16, kind="ExternalInput")
    out = nc.dram_tensor("out", [NT, C], BF16, kind="ExternalOutput")

    with tile.TileContext(nc) as tc:
        with (
            tc.tile_pool(name="const", bufs=1) as const,
            tc.tile_pool(name="res", bufs=1) as res,
        ):
            # --- constants (built in f32, cast to bf16 once) ---
            ident = const.tile([128, 128], BF16, tag="ident")
            # sliding causal mask: wmask[k, u] = 1 iff k <= u - 512; a crossing
            # tile r multiplies by wmask[:, 512-128r : 1024-128r]
            wmask = const.tile([128, 1024], BF16, tag="wmask")
            ones_col = const.tile([128, 1], BF16, tag="ones_col")
            # indicator lhsT per q-block: Emat[:, qb, j] selects den row qb
            # (head 0) for j<64 and row 4+qb (head 1) for j>=64, so one matmul
            # broadcasts both heads' reciprocals into a [128, 512] tile
            emat = const.tile([8, 4, 128], BF16, tag="emat")
            with tc.tile_pool(name="cstage", bufs=1) as cstage:
                ident_s = cstage.tile([128, 128], F32, tag="ident_s")
                make_identity(nc, ident_s[:])
                nc.vector.tensor_copy(ident[:], ident_s[:])

                wmask_s = cstage.tile([128, 1024], F32, tag="wmask_s")
                nc.gpsimd.memset(wmask_s[:], 0.0)
                nc.gpsimd.affine_select(
                    out=wmask_s[:],
                    in_=wmask_s[:],
                    compare_op=mybir.AluOpType.is_gt,
                    fill=1.0,
                    base=512,
                    # keep 0 where (512 + k - u) > 0, fill 1 where k <= u - 512
                    pattern=[[-1, 1024]],
                    channel_multiplier=1,
                )
                nc.vector.tensor_copy(wmask[:], wmask_s[:])

                ones_s = cstage.tile([128, 1], F32, tag="ones_s")
                nc.gpsimd.memset(ones_s[:], 1.0)
                nc.vector.tensor_copy(ones_col[:], ones_s[:])

            bq_t = const.tile([DPC, 1], F32, tag="bq")
            bk_t = const.tile([DPC, 1], F32, tag="bk")
            bv_t = const.tile([DPC, 1], F32, tag="bv")
            nc.sync.dma_start(bq_t[:], bq[:])
            nc.sync.dma_start(bk_t[:], bk[:])
            nc.sync.dma_start(bv_t[:], bv[:])

            # weights -> SBUF directly in bf16 (cast on host)
            wq_r = const.tile([128, CH, DPC], BF16, tag="wq_r")
            wk_r = const.tile([128, CH, DPC], BF16, tag="wk_r")
            wv_r = const.tile([128, CH, DPC], BF16, tag="wv_r")
            wp_r = const.tile([DPC, C], BF16, tag="wp_r")
            for w_in, w_dst in ((wq, wq_r), (wk, wk_r), (wv, wv_r)):
                w_re = w_in.rearrange("(c p) n -> p c n", p=128)
                nc.sync.dma_start(w_dst[:], w_re[:])
            nc.sync.dma_start(wp_r[:], wp[:])
            nc.sync.dma_start(emat[:], emat_in[:])

            # --- residents ---
            qT = res.tile([DPC, NT], BF16, tag="qT")
            kT = res.tile([DPC, NT], BF16, tag="kT")
            vT = res.tile([DPC, NT], BF16, tag="vT")
            yT = res.tile([DPC, NT], BF16, tag="yT")

            xT_re = xT.rearrange("(c p) t -> p c t", p=128)
            n_ktiles = T // KT  # 16

            with (
                tc.tile_pool(name="xpool", bufs=3) as xpool,
                tc.tile_pool(name="vpool", bufs=34) as vpool,
                tc.tile_pool(name="epool", bufs=CHUNK + 3) as epool,
                tc.tile_pool(name="dpool", bufs=2) as dpool,
                tc.tile_pool(name="opool", bufs=6) as opool,
                tc.tile_pool(name="q_psum", bufs=2, space="PSUM") as q_psum,
                tc.tile_pool(name="s_psum", bufs=2, space="PSUM") as s_psum,
                tc.tile_pool(name="y_psum", bufs=2, space="PSUM") as y_psum,
                tc.tile_pool(name="p_psum", bufs=2, space="PSUM") as p_psum,
            ):
                xs_tiles = {}

                def dma_x(tt):
                    if tt >= NT // QB or tt in xs_tiles:
                        return
                    xs = xpool.tile([128, CH, QB], BF16, tag="xs", name=f"xs{tt}")
                    nc.sync.dma_start(
                        xs[:], xT_re[:, :, tt * QB : (tt + 1) * QB]
                    )
                    xs_tiles[tt] = xs

                def qkv_tile(tt):
                    """Project one 512-token tile into qT/kT/vT; prefetch x."""
                    dma_x(tt + 1)
                    xs = xs_tiles.pop(tt)
                    ts_ = slice(tt * QB, (tt + 1) * QB)
                    psq = q_psum.tile([128, QB], F32, tag="qkv", name=f"psq{tt}")
                    psk = q_psum.tile([128, QB], F32, tag="qkv", name=f"psk{tt}")
                    psv = q_psum.tile([128, QB], F32, tag="qkv", name=f"psv{tt}")
                    for ps, w_r in ((psq, wq_r), (psk, wk_r), (psv, wv_r)):
                        for c in range(CH):
                            nc.tensor.matmul(
                                ps[:], w_r[:, c, :], xs[:, c, :],
                                start=(c == 0), stop=(c == CH - 1),
                            )
                    # copy out of PSUM (+bias; q also scaled by 1/sqrt(dh))
                    nc.scalar.activation(qT[:, ts_], psq[:], AF.Identity, bias=bq_t[:], scale=SCALE)
                    nc.vector.tensor_scalar_add(kT[:, ts_], psk[:], bk_t[:])
                    nc.vector.tensor_scalar_add(vT[:, ts_], psv[:], bv_t[:])

                # per-batch state
                vts_all = {}   # b -> list of 16 [128, 130] tiles
                den_all = {}   # b -> (denw, den)

                def vts_half(b, half):
                    """Transpose 8 V token-tiles (both heads at once).

                    v tile layout [128 tok, 130]: cols 0-63 head0 dims, col 64
                    ones, cols 65-128 head1 dims, col 129 ones. Head hl's PV
                    lhsT is v[:, 65*hl : 65*hl+65] -> psum rows 0-63 = y,
                    row 64 = denominator.
                    """
                    cb = b * T
                    vts = vts_all.setdefault(b, [None] * n_ktiles)
                    for kt in range(half * 8, half * 8 + 8):
                        pt = s_psum.tile([128, 512], BF16, tag="s", name=f"pt{b}_{kt}")
                        nc.tensor.transpose(
                            pt[:, :128],
                            vT[:, cb + kt * KT : cb + (kt + 1) * KT],
                            ident[:],
                        )
                        v = vpool.tile([128, 130], BF16, tag="v", name=f"v{b}_{kt}")
                        nc.vector.tensor_copy(v[:, 0:64], pt[:, 0:64])
                        nc.vector.tensor_copy(v[:, 65:129], pt[:, 64:128])
                        nc.vector.tensor_copy(v[:, 64:65], ones_col[:])
                        nc.vector.tensor_copy(v[:, 129:130], ones_col[:])
                        vts[kt] = v

                def sp_unit(b, hl, qb):
                    """Scores + exp + P@V for one (head, q-block)."""
                    cb = b * T
                    rb = hl * DH
                    vts = vts_all[b]
                    qs = slice(cb + qb * QB, cb + (qb + 1) * QB)
                    py = y_psum.tile([128, QB], F32, tag="py", name=f"py{b}_{hl}_{qb}")
                    nkt = (qb + 1) * (QB // KT)
                    for k0 in range(0, nkt, CHUNK):
                        kts = range(k0, min(k0 + CHUNK, nkt))
                        exs = {}
                        # scores + exp for this chunk
                        for kt in kts:
                            ps = s_psum.tile([128, QB], F32, tag="s", name=f"ps{kt}")
                            nc.tensor.matmul(
                                ps[:],
                                kT[rb : rb + DH, cb + kt * KT : cb + (kt + 1) * KT],
                                qT[rb : rb + DH, qs],
                                start=True,
                                stop=True,
                            )
                            ex = epool.tile([128, QB], BF16, tag="ex", name=f"ex{kt}")
                            nc.scalar.activation(ex[:], ps[:], AF.Exp)
                            r = kt - qb * (QB // KT)
                            if r >= 0:
                                # diagonal-crossing tile: zero out k > q
                                nc.vector.tensor_mul(
                                    ex[:], ex[:],
                                    wmask[:, 512 - r * KT : 1024 - r * KT],
                                )
                            exs[kt] = ex
                        # grouped P@V accumulation for this chunk
                        for kt in kts:
                            nc.tensor.matmul(
                                py[: DH + 1],
                                vts[kt][:, 65 * hl : 65 * hl + 65],
                                exs[kt][:],
                                start=(kt == 0),
                                stop=(kt == nkt - 1),
                            )
                    # stash unnormalized y and the denominator row
                    denw, _den = den_all[b]
                    p = hl * 4 + qb
                    nc.vector.tensor_copy(
                        denw[:, p * QB : (p + 1) * QB], py[DH : DH + 1, :]
                    )
                    nc.vector.tensor_copy(yT[rb : rb + DH, qs], py[:DH, :])

                def den_prep(b):
                    """Scatter denominator rows to 8 partitions + reciprocal."""
                    denw, den = den_all[b]
                    for p in range(8):
                        nc.sync.dma_start(
                            den[p : p + 1, :], denw[:, p * QB : (p + 1) * QB]
                        )
                    rec = dpool.tile([8, QB], BF16, tag="rec", name=f"rec{b}")
                    with nc.allow_low_precision(reason="softmax denom recip to bf16"):
                        nc.vector.reciprocal(rec[:], den[:])
                    return rec

                def norm(b, rec):
                    cb = b * T
                    for qb in range(T // QB):
                        qs = slice(cb + qb * QB, cb + (qb + 1) * QB)
                        pb = p_psum.tile([128, 512], F32, tag="p", name=f"pb{b}_{qb}")
                        nc.tensor.matmul(
                            pb[:, :QB], emat[:, qb, :], rec[:],
                            start=True, stop=True,
                        )
                        nc.vector.tensor_mul(yT[:, qs], yT[:, qs], pb[:, :QB])

                def proj_quarter(b, i):
                    """Output projection for 4 of the batch's 16 token tiles."""
                    cb = b * T
                    for tt in range(i * 4, i * 4 + 4):
                        trow = cb + tt * 128
                        for half in range(2):
                            pp = p_psum.tile([128, 512], F32, tag="p", name=f"pp{b}_{tt}_{half}")
                            nc.tensor.matmul(
                                pp[:],
                                yT[:, trow : trow + 128],
                                wp_r[:, half * 512 : (half + 1) * 512],
                                start=True,
                                stop=True,
                            )
                            os_ = opool.tile([128, 512], BF16, tag="os", name=f"os{b}_{tt}_{half}")
                            # alternate copy engine: ACT carries exp, DVE the rest
                            if (tt + half) % 2 == 0:
                                nc.vector.tensor_copy(os_[:], pp[:])
                            else:
                                nc.scalar.copy(os_[:], pp[:])
                            nc.sync.dma_start(
                                out[trow : trow + 128, half * 512 : (half + 1) * 512],
                                os_[:],
                            )

                # ================= emission schedule =================
                dma_x(0)
                for tt in range(TPB):          # qkv for batch 0
                    qkv_tile(tt)
                for b in range(B):
                    denw = dpool.tile([1, 8 * QB], F32, tag="denw", name=f"denw{b}")
                    den = dpool.tile([8, QB], F32, tag="den", name=f"den{b}")
                    den_all[b] = (denw, den)
                    if b == 0:
                        vts_half(0, 0)
                        vts_half(0, 1)
                    # interleave next batch's qkv tiles into this batch's
                    # attention stream (power smoothing + overlap)
                    sps = [(hl, qb) for hl in range(HPC) for qb in range(T // QB)]
                    qnext = list(range((b + 1) * TPB, (b + 2) * TPB)) if b + 1 < B else []
                    qpos = {0: 0, 1: 2, 2: 4, 3: 6}  # after sp index i emit qkv tile
                    for i, (hl, qb) in enumerate(sps):
                        sp_unit(b, hl, qb)
                        for j, pos in qpos.items():
                            if pos == i and j < len(qnext):
                                qkv_tile(qnext[j])
                    rec = den_prep(b)
                    # fill the reciprocal latency gap with next batch's V transposes
                    if b + 1 < B:
                        vts_half(b + 1, 0)
                        vts_half(b + 1, 1)
                        vts_all.pop(b, None)
                    norm(b, rec)
                    for i in range(4):
                        proj_quarter(b, i)

    nc.compile()
    return nc


def _get_nc():
    global _CACHED_NC
    if _CACHED_NC is None:
        _CACHED_NC = _build()
    return _CACHED_NC


def kernel(x, W_qkv, b_qkv, W_proj, b_proj, _trace=False, _core_ids=None):
    global LAST_RESULT
    x = np.asarray(x, dtype=np.float32)
    W_qkv = np.asarray(W_qkv, dtype=np.float32)
    b_qkv = np.asarray(b_qkv, dtype=np.float32)
    W_proj = np.asarray(W_proj, dtype=np.float32)
    b_proj = np.asarray(b_proj, dtype=np.float32)

    xT = np.ascontiguousarray(x.reshape(NT, C).T).astype(BF16_NP)
    W_qkv_b = W_qkv.astype(BF16_NP)
    W_proj_b = W_proj.astype(BF16_NP)
    emat_np = np.zeros((8, 4, 128), dtype=BF16_NP)
    for qb in range(4):
        emat_np[qb, qb, :DH] = 1.0
        emat_np[4 + qb, qb, DH:] = 1.0
    core_ids = list(range(N_CORES)) if _core_ids is None else _core_ids
    in_maps = []
    for core in range(len(core_ids)):
        s = slice(core * DPC, (core + 1) * DPC)
        in_maps.append(
            {
                "xT": xT,
                "wq": np.ascontiguousarray(W_qkv_b[:, 0 * C + core * DPC : 0 * C + (core + 1) * DPC]),
                "wk": np.ascontiguousarray(W_qkv_b[:, 1 * C + core * DPC : 1 * C + (core + 1) * DPC]),
                "wv": np.ascontiguousarray(W_qkv_b[:, 2 * C + core * DPC : 2 * C + (core + 1) * DPC]),
                # device computes qT = psq*SCALE + bias, so pre-scale the q bias
                "bq": np.ascontiguousarray(b_qkv[0 * C + core * DPC : 0 * C + (core + 1) * DPC, None]) * np.float32(SCALE),
                "bk": np.ascontiguousarray(b_qkv[1 * C + core * DPC : 1 * C + (core + 1) * DPC, None]),
                "bv": np.ascontiguousarray(b_qkv[2 * C + core * DPC : 2 * C + (core + 1) * DPC, None]),
                "wp": np.ascontiguousarray(W_proj_b[s, :]),
                "emat": emat_np,
            }
        )

    nc = _get_nc()
    res = run_bass_kernel_spmd(nc, in_maps, core_ids, trace=_trace)
    LAST_RESULT = res

    acc = np.zeros((NT, C), dtype=np.float64)
    for r in res.results:
        acc += r["out"].astype(np.float64)
    acc += b_proj.astype(np.float64)
    return acc.reshape(B, T, C).astype(np.float32)
